# revision 9
# baseline (speedup 1.0000x reference)
"""Trainium2 Bass kernel for nn_CoevolutionAnalyzer (pairwise-MLP coevolution scores).

Math (per batch b):
    g = domain * evo                         [512, 128]
    a = g @ W1[:128], c = g @ W1[128:]       [512, 128]
    h_ij  = relu(a_i + c_j + b1)             [128]
    z2    = W2.T h_ij + b2 ; h2 = relu(z2)   [64]
    s_ij  = sigmoid(W3.h2 + b3)
    out   = triu(s,1) + triu(s,1).T

Sharding (8 cores, single SPMD program):
    Upper triangle is computed block-wise: j-block B of 64 columns is needed by
    rows i < 64*B+64, i.e. row i needs j in [64*floor(i/64), 512) (padded to the
    block grid; the pad is masked on the host via triu). Every core takes 8 rows
    of every 64-row block (rows 64*b + 8*k .. +8) for both batches, so all cores
    run the identical instruction stream; only DMA'd data differs. The i-side
    inputs are host-gathered per core so on-device column indices are
    core-independent.
"""

import os

import numpy as np
from ml_dtypes import bfloat16 as bf16_np

import concourse.bass as bass
import concourse.tile as tile
from concourse import bacc, mybir
from concourse.bass_utils import run_bass_kernel_spmd

B = 2
N = 512
D = 128
NB = 8          # number of 64-row j-blocks
BLK = N // NB   # 64
RPB = 8         # rows per core per block
F32 = mybir.dt.float32
F32R = mybir.dt.float32r
BF16 = mybir.dt.bfloat16
AF = mybir.ActivationFunctionType
ALU = mybir.AluOpType

LAST_RESULT = None  # set by kernel(); test harness reads exec_time_ns


def _build():
    nc = bacc.Bacc("TRN2", target_bir_lowering=False, debug=False, num_devices=8)

    # Per-core inputs. dom_i/evo_i hold only this core's 64 i-rows per batch
    # (transposed, permuted); dom_j/evo_j hold all 512 positions per batch.
    dom_i = nc.declare_dram_parameter("dom_i", [D, B * NB * RPB], F32, isOutput=False)
    evo_i = nc.declare_dram_parameter("evo_i", [D, B * NB * RPB], F32, isOutput=False)
    dom_j = nc.declare_dram_parameter("dom_j", [D, B * N], F32, isOutput=False)
    evo_j = nc.declare_dram_parameter("evo_j", [D, B * N], F32, isOutput=False)
    w1a = nc.declare_dram_parameter("w1a", [D, D], F32R, isOutput=False)
    w1b = nc.declare_dram_parameter("w1b", [D, D], F32R, isOutput=False)
    b1 = nc.declare_dram_parameter("b1", [D, 1], F32, isOutput=False)
    w2 = nc.declare_dram_parameter("w2", [D, D // 2], BF16, isOutput=False)
    b2s = nc.declare_dram_parameter("b2s", [D, 1], F32, isOutput=False)
    w3s = nc.declare_dram_parameter("w3s", [D, 32], BF16, isOutput=False)
    b3t = nc.declare_dram_parameter("b3t", [D, 1], F32, isOutput=False)
    out = nc.declare_dram_parameter("out", [B * NB * RPB, N], F32, isOutput=True)

    with tile.TileContext(nc) as tc:
        with (
            tc.tile_pool(name="singles", bufs=1) as singles,
            tc.tile_pool(name="per_batch", bufs=2) as per_batch,
            tc.tile_pool(name="hpool", bufs=6) as hpool,
            tc.tile_pool(name="h2pool", bufs=3) as h2pool,
            tc.tile_pool(name="sigpool", bufs=2) as sigpool,
            tc.tile_pool(name="psz", bufs=3, space="PSUM") as psz_pool,
            tc.tile_pool(name="pss", bufs=2, space="PSUM") as pss_pool,
            tc.tile_pool(name="pset", bufs=1, space="PSUM") as pset_pool,
        ):
            s_di = singles.tile([D, B * NB * RPB], F32)
            s_ei = singles.tile([D, B * NB * RPB], F32)
            s_dj = singles.tile([D, B * N], F32)
            s_ej = singles.tile([D, B * N], F32)
            s_w1a = singles.tile([D, D], F32R)
            s_w1b = singles.tile([D, D], F32R)
            s_b1 = singles.tile([D, 1], F32)
            s_w2 = singles.tile([D, D // 2], BF16)
            s_b2s = singles.tile([D, 1], F32)
            s_w3s = singles.tile([D, 32], BF16)
            s_b3 = singles.tile([D, 1], F32)
            nc.sync.dma_start(out=s_b3, in_=b3t[:])
            nc.sync.dma_start(out=s_w1a, in_=w1a[:])
            nc.sync.dma_start(out=s_w1b, in_=w1b[:])
            nc.sync.dma_start(out=s_b1, in_=b1[:])
            nc.sync.dma_start(out=s_w2, in_=w2[:])
            nc.sync.dma_start(out=s_b2s, in_=b2s[:])
            nc.sync.dma_start(out=s_w3s, in_=w3s[:])
            nc.sync.dma_start(out=s_di, in_=dom_i[:])
            nc.sync.dma_start(out=s_ei, in_=evo_i[:])
            # batch-0 halves first so compute can start earlier
            nc.sync.dma_start(out=s_dj[:, :N], in_=dom_j[:, :N])
            nc.sync.dma_start(out=s_ej[:, :N], in_=evo_j[:, :N])
            nc.sync.dma_start(out=s_dj[:, N:], in_=dom_j[:, N:])
            nc.sync.dma_start(out=s_ej[:, N:], in_=evo_j[:, N:])

            for q in range(B):
                qi = q * NB * RPB
                qj = q * N
                # --- per-batch setup: gT, aT(+b1), cT ---
                gti = per_batch.tile([D, NB * RPB], F32R, tag="gti")
                nc.vector.tensor_mul(
                    gti, s_di[:, qi : qi + NB * RPB], s_ei[:, qi : qi + NB * RPB]
                )
                gtj = per_batch.tile([D, N], F32R, tag="gtj")
                nc.gpsimd.tensor_mul(
                    gtj, s_dj[:, qj : qj + N], s_ej[:, qj : qj + N]
                )
                ps_a = pset_pool.tile([D, NB * RPB], F32, tag="ps_a")
                nc.tensor.matmul(
                    ps_a[:], s_w1a[:], gti[:]
                )
                ps_c = pset_pool.tile([D, N], F32, tag="ps_c")
                nc.tensor.matmul(
                    ps_c[:], s_w1b[:], gtj[:]
                )
                abt = per_batch.tile([D, NB * RPB], F32, tag="abt")
                nc.vector.tensor_scalar_add(abt, ps_a[:], s_b1[:, 0:1])
                ct = per_batch.tile([D, N], BF16, tag="ct")
                nc.scalar.copy(ct, ps_c[:])

                # --- j-block loop ---
                for b in range(NB):
                    j0 = BLK * b
                    L = N - j0
                    G = min(4, N // L)  # pairs per PSUM bank round
                    R = 4 // G          # rounds per block (4 pairs total)
                    pss = pss_pool.tile([D, N], F32, tag="pss")
                    for r in range(R):
                        hs0 = hpool.tile([D, N], BF16, tag="hs0")
                        hs1 = hpool.tile([D, N], BF16, tag="hs1")
                        for g in range(G):
                            u = r * G + g  # pair index within block
                            for half in range(2):
                                m = 2 * u + half  # local row 0..7
                                hs = hs1 if half else hs0
                                nc.vector.tensor_scalar(
                                    out=hs[:, g * L : (g + 1) * L],
                                    in0=ct[:, j0:N],
                                    scalar1=abt[:, b * RPB + m : b * RPB + m + 1],
                                    scalar2=0.0,
                                    op0=ALU.add,
                                    op1=ALU.max,
                                )
                        GL = G * L
                        psz = psz_pool.tile([D, N], F32, tag="psz")
                        nc.tensor.matmul(
                            psz[0 : D // 2, :GL],
                            s_w2[:],
                            hs0[:, :GL],
                            tile_position=(0, 0),
                        )
                        nc.tensor.matmul(
                            psz[D // 2 : D, :GL],
                            s_w2[:],
                            hs1[:, :GL],
                            tile_position=(0, 64),
                        )
                        h2 = h2pool.tile([D, N], BF16, tag="h2")
                        nc.scalar.activation(
                            out=h2[:, :GL],
                            in_=psz[:, :GL],
                            func=AF.Relu,
                            bias=s_b2s[:, 0:1],
                        )
                        for g in range(G):
                            u = r * G + g
                            nc.tensor.matmul(
                                pss[32 * u : 32 * u + 32, :L],
                                s_w3s[:],
                                h2[:, g * L : (g + 1) * L],
                                tile_position=(0, 32 * u),
                            )
                    sig = sigpool.tile([D, N], F32, tag="sig")
                    nc.scalar.activation(
                        out=sig[:, :L],
                        in_=pss[:, :L],
                        func=AF.Sigmoid,
                        bias=s_b3[:, 0:1],
                    )
                    rowbase = q * NB * RPB + b * RPB
                    for u in range(4):
                        nc.sync.dma_start(
                            out=out[rowbase + 2 * u : rowbase + 2 * u + 2, j0:N],
                            in_=sig[32 * u : 32 * u + 2, :L],
                        )

    nc.compile()
    return nc


def kernel(
    domain_features,
    evolutionary_features,
    W1,
    b1,
    W2,
    b2,
    W3,
    b3,
):
    global LAST_RESULT
    dom = np.ascontiguousarray(np.asarray(domain_features, dtype=np.float32))
    evo = np.ascontiguousarray(np.asarray(evolutionary_features, dtype=np.float32))
    W1 = np.asarray(W1, dtype=np.float32)
    b1 = np.asarray(b1, dtype=np.float32)
    W2 = np.asarray(W2, dtype=np.float32)
    b2 = np.asarray(b2, dtype=np.float32)
    W3 = np.asarray(W3, dtype=np.float32)
    b3 = np.asarray(b3, dtype=np.float32)

    nc = _build()

    # host-side shard prep
    w1a = np.ascontiguousarray(W1[:D])
    w1b = np.ascontiguousarray(W1[D:])
    b1c = b1.reshape(D, 1)
    b2s = np.concatenate([b2, b2]).reshape(D, 1)
    w3s = np.zeros((D, 32), np.float32)
    w3s[: D // 2, 0] = W3[:, 0]
    w3s[D // 2 :, 1] = W3[:, 0]

    dom_jT = np.ascontiguousarray(
        np.concatenate([dom[q].T for q in range(B)], axis=1)
    )  # [128, 1024]
    evo_jT = np.ascontiguousarray(
        np.concatenate([evo[q].T for q in range(B)], axis=1)
    )

    in_maps = []
    for k in range(8):
        rows = np.concatenate(
            [BLK * bb + RPB * k + np.arange(RPB) for bb in range(NB)]
        )  # 64 rows per batch
        dom_iT = np.ascontiguousarray(
            np.concatenate([dom[q][rows].T for q in range(B)], axis=1)
        )  # [128, 128]
        evo_iT = np.ascontiguousarray(
            np.concatenate([evo[q][rows].T for q in range(B)], axis=1)
        )
        in_maps.append(
            {
                "dom_i": dom_iT,
                "evo_i": evo_iT,
                "dom_j": dom_jT,
                "evo_j": evo_jT,
                "w1a": w1a,
                "w1b": w1b,
                "b1": b1c,
                "w2": np.ascontiguousarray(W2).astype(bf16_np),
                "b2s": b2s,
                "w3s": w3s.astype(bf16_np),
                "b3t": np.full((D, 1), float(b3[0]), np.float32),
            }
        )

    trace = os.environ.get("KERNEL_TRACE", "0") == "1"
    res = run_bass_kernel_spmd(nc, in_maps, core_ids=list(range(8)), trace=trace)
    LAST_RESULT = res

    S = np.zeros((B, N, N), np.float32)
    for k in range(8):
        o = res.results[k]["out"]  # [128, 512]
        for q in range(B):
            for bb in range(NB):
                r0 = BLK * bb + RPB * k
                S[q, r0 : r0 + RPB, :] = o[q * NB * RPB + bb * RPB :][:RPB]
    upper = np.triu(S, 1)
    return (upper + upper.transpose(0, 2, 1)).astype(np.float32)
